# revision 18
# baseline (speedup 1.0000x reference)
"""Trainium2 Bass kernel for nn_GCNNet (3-layer GCNConv+BN+ReLU, JK concat),
distributed over 8 NeuronCores.

Strategy (scatter-by-source + ReduceScatter):
  Nodes are partitioned across cores (round-robin by out-degree).  Each core
  keeps its own nodes' features resident in SBUF, computes the per-layer
  messages m = (o @ W_l) * dinv locally, and processes the edges whose SOURCE
  it owns: it gathers m rows from its own DRAM (dma_gather), forms one-hot
  selection matrices per 128-edge chunk (DVE is_equal vs iota), and uses PE
  matmuls to produce partial aggregates for ALL destination tiles (392 global
  tiles, capacity-packed so the chunk grid is identical on every core).
  Partial aggregates are summed and re-sharded with two ReduceScatter
  collectives (front/back halves, so the first overlaps the tail of the
  compute).  Self-loop terms never touch DRAM: they are added post-RS via a
  PE transpose of the local message block.  BN statistics use a small
  AllGather; BN scale/shift + ReLU run on the scalar engine.

  Collective cost goes from ~250us/layer (AllGather of the full message
  replica) to ~70us/layer (RS output is only the local shard).

kernel(**inputs) takes the FULL inputs and returns the FULL [N, 512] output.
"""

import os

# ReduceScatter through the stock NRT path is broken on this deployment
# (NRT_EXEC_UNIT_UNRECOVERABLE); the RDH customcomms lowering works.
os.environ.setdefault("TRNINF_ENABLE_CUSTOMCOMMS_RDH_RS", "1")

import ml_dtypes as _ml_dtypes
import numpy as np

import concourse.bacc as bacc
import concourse.bass as bass
import concourse.mybir as mybir
import concourse.tile as tile
from concourse.library_config import mlp as mlp_library

F32 = mybir.dt.float32
BF16 = mybir.dt.bfloat16
I16 = mybir.dt.int16
AX = mybir.AxisListType
OP = mybir.AluOpType
ACTF = mybir.ActivationFunctionType


class Cfg:
    pass


# ----------------------------------------------------------------------------
# Host preprocessing
# ----------------------------------------------------------------------------

def _pack_tiles(dv, NT, m4=20, m3=16, m2=12):
    """Pack n nodes (rows of dv, [n, C] in-edge demand vectors) into NT tiles
    of <=128 nodes, minimizing sum_t ceil(max_c load_tc / 128).  Uses a
    bimodal level schedule (a few big L3/L4 tiles soak up the heavy nodes so
    most tiles stay under the 256-edge L2 cap).  Returns tile_of [n]."""
    n, Cc = dv.shape
    tot = dv.sum()
    ordern = np.argsort(-dv.sum(1), kind="stable")
    best = None
    for a in range(0, 5):
        for b in range(0, 12):
            x = NT - a - b
            if x < 0:
                continue
            cap = (a * (512 - m4) + b * (384 - m3) + x * (256 - m2)) * Cc
            if cap >= tot:
                s = 4 * a + 3 * b + 2 * x
                if best is None or s < best[0]:
                    best = (s, a, b)
    _, a, b = best
    nbig = a + b
    caps = np.array([512 - m4] * a + [384 - m3] * b, np.int64)
    loads = np.zeros((NT, Cc), np.int64)
    cnts = np.zeros(NT, np.int64)
    tile_of = np.empty(n, np.int64)
    light_nodes = []
    placed_big = 0
    for i in ordern:
        if placed_big < nbig * 128 and nbig > 0:
            lt = loads[:nbig] + dv[i]
            ok = (lt <= caps[:, None]).all(1) & (cnts[:nbig] < 128)
            if ok.any():
                fill = lt.max(1) / np.maximum(caps, 1)
                fill[~ok] = -1.0
                t = int(np.argmax(fill))
                tile_of[i] = t
                loads[t] += dv[i]
                cnts[t] += 1
                placed_big += 1
                continue
        light_nodes.append(i)
    g = NT - nbig
    light_nodes = np.array(light_nodes)
    for r0 in range(0, len(light_nodes), g):
        rnd = light_nodes[r0:r0 + g]
        used = np.zeros(g, bool)
        for i in rnd:
            lt = (loads[nbig:] + dv[i]).max(1).astype(np.float64)
            sc = lt + np.where(used, 1e9, 0) + \
                np.where(cnts[nbig:] >= 128, 1e12, 0)
            if sc.min() >= 1e12:           # light tiles full: spill to big
                lt2 = (loads + dv[i]).max(1).astype(np.float64)
                sc2 = lt2 + np.where(cnts >= 128, 1e12, 0)
                t = int(np.argmin(sc2))
            else:
                t = nbig + int(np.argmin(sc))
                used[t - nbig] = True
            tile_of[i] = t
            loads[t] += dv[i]
            cnts[t] += 1
    return tile_of


def preprocess(x, edge_index, Ws, gammas, betas, C=8, eps=1e-5):
    N, D = x.shape
    assert D == 128
    L = Ws.shape[0]
    E = edge_index.shape[1]

    NPC = (N + C - 1) // C           # nodes per core
    NT = (NPC + 127) // 128          # tiles per core (49)
    NTP = NT * 128                   # local slots per core (6272)

    src = edge_index[0].astype(np.int64)
    dst = edge_index[1].astype(np.int64)

    indeg = np.bincount(dst, minlength=N) + 1        # A + I in-degree
    dinv = (1.0 / np.sqrt(indeg.astype(np.float64))).astype(np.float32)
    outdeg = np.bincount(src, minlength=N) + 1

    # --- node -> owner core (round-robin by out-degree: balances streams) ---
    order = np.argsort(-outdeg, kind="stable")
    owner = np.empty(N, np.int64)
    owner[order] = np.arange(N) % C

    # --- per-node demand vectors (in-edges per src core, NO self loops) ----
    dvc = np.zeros((N, C), np.int64)
    np.add.at(dvc, (dst, owner[src]), 1)

    # --- per-owner-core packing into NT tiles -----------------------------
    slot_of = np.empty(N, np.int64)                 # global slot
    for co in range(C):
        nodes = np.where(owner == co)[0]
        tile_of = _pack_tiles(dvc[nodes], NT)
        # positions within tile
        pos = np.zeros(len(nodes), np.int64)
        for t in range(NT):
            sel = np.where(tile_of == t)[0]
            pos[sel] = np.arange(len(sel))
            assert len(sel) <= 128
        slot_of[nodes] = co * NTP + tile_of * 128 + pos

    node_of_slot = np.full(C * NTP, -1, np.int64)
    node_of_slot[slot_of] = np.arange(N)

    # --- per-global-tile chunk counts (uniform across cores) --------------
    NGT = C * NT                                    # 392 global tiles
    # load per (src core, global tile)
    gt_of_dst = slot_of[dst] // 128
    loads = np.zeros((C, NGT), np.int64)
    np.add.at(loads, (owner[src], gt_of_dst), 1)
    CH = np.maximum(np.ceil(loads.max(0) / 128).astype(np.int64), 1)  # [NGT]

    # --- processing order: (half, r, tl); A = tl<NA, B = tl>=NA -----------
    NA = min(36, NT - 4)                            # A tiles per dst core
    NB = NT - NA                                    # 25 B tiles
    gt_list = []
    for half in (0, 1):
        for r in range(C):
            tls = range(0, NA) if half == 0 else range(NA, NT)
            for tl in tls:
                gt_list.append(r * NT + tl)
    gt_order = np.array(gt_list)                    # processing order
    # chunk column base per gt (in processing order)
    chbase = np.zeros(NGT, np.int64)
    acc = 0
    for gt in gt_order:
        chbase[gt] = acc
        acc += CH[gt]
    CHT = acc                                       # total chunks

    # --- per-core grid data: gather idx + dstb ----------------------------
    # m_d storage row for local slot s: rows are p-major: row = (s%128)*NT + s//128
    idx_all = np.zeros((C, CHT * 128), np.int64)
    dstb_all = np.full((C, CHT, 128), -1.0, np.float32)
    e_src_slot = slot_of[src] - owner[src] * NTP    # local src slot
    e_row = (e_src_slot % 128) * NT + e_src_slot // 128   # m_d row
    e_dstpos = slot_of[dst] % 128
    e_gt = gt_of_dst
    e_core = owner[src]
    # rank of edge within its (core, gt) cell
    key = e_core * NGT + e_gt
    eorder = np.argsort(key, kind="stable")
    key_s = key[eorder]
    first = np.r_[True, key_s[1:] != key_s[:-1]]
    starts = np.where(first)[0]
    seg_ids = np.cumsum(first) - 1
    rank = np.arange(len(key_s)) - starts[seg_ids]
    cs = e_core[eorder]
    gts = e_gt[eorder]
    assert np.all(rank < CH[gts] * 128), "cell overflow"
    gpos = chbase[gts] * 128 + rank
    idx_all[cs, gpos] = e_row[eorder]
    dstb_all[cs, gpos // 128, gpos % 128] = e_dstpos[eorder]

    # wrapped idx layout: element j -> [j%16, j//16]; tiled to 128 partitions
    idx16 = idx_all.reshape(C, -1, 16).transpose(0, 2, 1).astype(np.int16)
    idx16 = np.tile(idx16, (1, 8, 1))               # [C, 128, CHT*8]
    # dstb SBUF layout: [128 rows, CHT cols]
    dstb = dstb_all.transpose(0, 2, 1).astype(_ml_dtypes.bfloat16)

    # --- calls: group consecutive tiles (processing order), <=48 chunks ---
    MAXCH = 48
    calls = []          # list of (ch0, nch, tiles=[(gt, CH_gt, bank_end?)...])
    cur_tiles = []
    cur_ch = 0
    cur_c0 = 0
    pos_in_order = 0
    for oi, gt in enumerate(gt_order):
        half_end = (oi == C * NA - 1) or (oi == NGT - 1)
        if cur_ch + CH[gt] > MAXCH and cur_tiles:
            calls.append((cur_c0, cur_ch, cur_tiles))
            cur_c0 += cur_ch
            cur_ch = 0
            cur_tiles = []
        cur_tiles.append(int(gt))
        cur_ch += int(CH[gt])
        if half_end:
            calls.append((cur_c0, cur_ch, cur_tiles))
            cur_c0 += cur_ch
            cur_ch = 0
            cur_tiles = []
    assert cur_ch == 0 and sum(c[1] for c in calls) == CHT

    # --- self-loop one-hot source: selfb[p, tl] = p if slot real else -1 --
    # (per owner core; real slots depend on that core's packing)
    real_slot = node_of_slot >= 0                   # [C*NTP]

    iota = np.tile(np.arange(128, dtype=np.float32)[None, :], (128, 1))

    dinv_slot = np.zeros(C * NTP, np.float32)
    dinv_slot[slot_of] = dinv

    per_core = []
    for c in range(C):
        sl = slice(c * NTP, (c + 1) * NTP)
        x_fm = np.zeros((128, NTP), np.float32)
        vs = slot_of[owner == c] - c * NTP
        x_fm[:, vs] = x[owner == c].T
        dnm = dinv_slot[sl].reshape(NT, 128).T.copy()   # [128 pos, NT tile]
        d = {
            "x_fm": x_fm,
            "dinv_fm": np.tile(dinv_slot[sl][None, :], (128, 1)),
            "dinv_nm": dnm,
            "idx16": idx16[c],
            "dstb": dstb[c],
            "iota": iota.astype(_ml_dtypes.bfloat16),
            "selfb": np.where(real_slot[sl].reshape(NT, 128),
                              np.arange(128)[None, :], -1.0
                              ).T.astype(_ml_dtypes.bfloat16),
            "Ws": Ws.astype(np.float32),
            "gammaT": gammas.T.astype(np.float32).copy(),
            "betaT": betas.T.astype(np.float32).copy(),
        }
        per_core.append(d)

    cfg = Cfg()
    cfg.N, cfg.D, cfg.L, cfg.C, cfg.E = N, D, L, C, E
    cfg.NPC, cfg.NT, cfg.NTP = NPC, NT, NTP
    cfg.NA, cfg.NB = NA, NB
    cfg.CH, cfg.CHT, cfg.chbase = CH, CHT, chbase
    cfg.calls = calls
    cfg.gt_order = gt_order
    cfg.eps = eps
    cfg.owner = owner
    cfg.slot_of = slot_of
    cfg.node_of_slot = node_of_slot
    return cfg, per_core


def assemble_output(cfg, x, core_outs):
    """core_outs: per core [L, 128, NTP] bf16 -> [N, (L+1)*128] fp32."""
    N, L, C, NTP = cfg.N, cfg.L, cfg.C, cfg.NTP
    out = np.empty((N, (L + 1) * 128), np.float32)
    out[:, :128] = x
    for c in range(C):
        slots = cfg.node_of_slot[c * NTP:(c + 1) * NTP]
        valid = slots >= 0
        nodes = slots[valid]
        o = np.asarray(core_outs[c], dtype=np.float32)
        for l in range(L):
            out[nodes, (l + 1) * 128:(l + 2) * 128] = o[l][:, valid].T
    return out


# ----------------------------------------------------------------------------
# Bass kernel
# ----------------------------------------------------------------------------

def build_nc(cfg, skip=()):
    NT, NTP, CHT, NA, NB = cfg.NT, cfg.NTP, cfg.CHT, cfg.NA, cfg.NB
    L, C = cfg.L, cfg.C
    CH, chbase = cfg.CH, cfg.chbase
    WA, WB = NA * 128, NB * 128          # A/B half widths (3072 / 3200)
    MAXCH = max(c[1] for c in cfg.calls)

    nc = bacc.Bacc("TRN2", target_bir_lowering=False, num_devices=C)

    x_fm_t = nc.dram_tensor("x_fm", [128, NTP], F32, kind="ExternalInput")
    dinv_fm_t = nc.dram_tensor("dinv_fm", [128, NTP], F32, kind="ExternalInput")
    dinv_nm_t = nc.dram_tensor("dinv_nm", [128, NT], F32, kind="ExternalInput")
    idx16_t = nc.dram_tensor("idx16", [128, CHT * 8], I16, kind="ExternalInput")
    dstb_t = nc.dram_tensor("dstb", [128, CHT], BF16, kind="ExternalInput")
    iota_t = nc.dram_tensor("iota", [128, 128], BF16, kind="ExternalInput")
    selfb_t = nc.dram_tensor("selfb", [128, NT], BF16, kind="ExternalInput")
    Ws_t = nc.dram_tensor("Ws", [L, 128, 128], F32, kind="ExternalInput")
    gammaT_t = nc.dram_tensor("gammaT", [128, L], F32, kind="ExternalInput")
    betaT_t = nc.dram_tensor("betaT", [128, L], F32, kind="ExternalInput")
    o_out_t = nc.dram_tensor("o_out", [L, 128, NTP], BF16,
                             kind="ExternalOutput")

    groups = [list(range(C))]
    inv_n = 1.0 / float(cfg.N)

    with tile.TileContext(nc) as tc:
        with (
            tc.tile_pool(name="persist", bufs=1) as pp,
            tc.tile_pool(name="gath", bufs=2) as gp,
            tc.tile_pool(name="sel", bufs=2) as sp,
            tc.tile_pool(name="stag", bufs=4) as stp,
            tc.tile_pool(name="pm", bufs=3, space="PSUM") as pmp,
            tc.tile_pool(name="bank", bufs=4, space="PSUM") as bkp,
            tc.tile_pool(name="dram", bufs=1, space="DRAM") as dp,
        ):
            o_fm = pp.tile([128, NTP], F32)
            m_sb = pp.tile([128, NT, 128], BF16)
            agg = pp.tile([128, NTP], F32)
            sq = pp.tile([128, max(WA, WB)], F32)
            rs_sb = pp.tile([128, NTP], BF16)
            dinv_fm = pp.tile([128, NTP], F32)
            dinv_nm = pp.tile([128, NT], F32)
            idx16 = pp.tile([128, CHT * 8], I16)
            dstb = pp.tile([128, CHT], BF16)
            iota = pp.tile([128, 128], BF16)
            selfb = pp.tile([128, NT], BF16)
            Sself = pp.tile([128, NT, 128], BF16)
            Wt = pp.tile([128, L, 128], F32)
            gammaT = pp.tile([128, L], F32)
            betaT = pp.tile([128, L], F32)
            stat4 = pp.tile([128, 4], F32)
            statr = pp.tile([128, C, 4], F32)
            stats2 = pp.tile([128, 4], F32)
            prm = pp.tile([128, 8], F32)

            m_d = dp.tile([NTP, 128], BF16, name="m_d")
            partA_d = dp.tile([C, 128, WA], BF16, name="partA")
            partB_d = dp.tile([C, 128, WB], BF16, name="partB")
            rsA_ds = [dp.tile([128, WA], BF16, name=f"rsA_{l}")
                      for l in range(L)]
            rsB_ds = [dp.tile([128, WB], BF16, name=f"rsB_{l}")
                      for l in range(L)]
            stat_in_ds = [dp.tile([128, 4], F32, name=f"stat_in_{l}")
                          for l in range(L)]
            stat_out_ds = [dp.tile([C, 128, 4], F32, addr_space="Shared",
                                   name=f"stat_out_{l}") for l in range(L)]

            # --- load phase ---------------------------------------------
            nc.gpsimd.load_library(mlp_library)
            nc.sync.dma_start(o_fm[:], x_fm_t[:])
            nc.sync.dma_start(dinv_fm[:], dinv_fm_t[:])
            nc.sync.dma_start(dinv_nm[:], dinv_nm_t[:])
            nc.sync.dma_start(idx16[:], idx16_t[:])
            nc.sync.dma_start(dstb[:], dstb_t[:])
            nc.sync.dma_start(iota[:], iota_t[:])
            nc.sync.dma_start(selfb[:], selfb_t[:])
            nc.sync.dma_start(Wt[:], Ws_t[:].rearrange("l k f -> k l f"))
            nc.sync.dma_start(gammaT[:], gammaT_t[:])
            nc.sync.dma_start(betaT[:], betaT_t[:])
            # self-loop one-hot blocks (layer-invariant, built once)
            nc.vector.tensor_tensor(
                Sself[:],
                selfb[:].unsqueeze(2).to_broadcast([128, NT, 128]),
                iota[:].unsqueeze(1).to_broadcast([128, NT, 128]),
                OP.is_equal)

            for l in range(L):
                rsA_d, rsB_d = rsA_ds[l], rsB_ds[l]
                stat_in_d, stat_out_d = stat_in_ds[l], stat_out_ds[l]

                # ---- head: m = (o @ W_l) * dinv[src] -------------------
                for b in range(NT):
                    pm = pmp.tile([128, 128], F32, name="pm")
                    nc.tensor.matmul(
                        pm[:], lhsT=o_fm[:, b * 128:(b + 1) * 128],
                        rhs=Wt[:, l, :], start=True, stop=True)
                    nc.scalar.activation(
                        m_sb[:, b, :], pm[:], ACTF.Copy,
                        scale=dinv_nm[:, b:b + 1])
                # node-major rows to DRAM (p-major storage: row = p*NT + b)
                nc.sync.dma_start(
                    m_d[:].rearrange("(p b) f -> p b f", p=128), m_sb[:])

                # ---- phase B: partial aggregates for all global tiles ---
                bank = None
                bank_q = 0
                bank_gt0 = -1
                for (c0, nch, tiles) in cfg.calls:
                    gb = gp.tile([128, MAXCH, 128], BF16, name="gb")
                    if "gather" not in skip:
                        nidx = nch * 128
                        nc.gpsimd.dma_gather(
                            gb[:, :nch, :], m_d[:],
                            idx16[:, c0 * 8:(c0 + nch) * 8],
                            nidx, nidx, 128, single_packet=False)
                    S = sp.tile([128, MAXCH, 128], BF16, name="S")
                    if "sbuild" not in skip:
                        nc.vector.tensor_tensor(
                            S[:, :nch, :],
                            dstb[:, c0:c0 + nch].unsqueeze(2)
                                .to_broadcast([128, nch, 128]),
                            iota[:].unsqueeze(1).to_broadcast([128, nch, 128]),
                            OP.is_equal)
                    for gt in tiles:
                        r, tl = gt // NT, gt % NT
                        q = (tl % NA if tl < NA else (tl - NA)) % 4
                        if q == 0:
                            bank = bkp.tile([128, 512], F32, name="bank")
                            bank_gt0 = gt
                        nch_t = int(CH[gt])
                        cb = int(chbase[gt]) - c0
                        for j in range(nch_t):
                            cc = cb + j
                            if "aggmm" not in skip:
                                nc.tensor.matmul(
                                    bank[:, q * 128:(q + 1) * 128],
                                    lhsT=gb[:, cc, :], rhs=S[:, cc, :],
                                    start=(j == 0), stop=(j == nch_t - 1))
                            else:
                                nc.tensor.matmul(
                                    bank[:, q * 128:(q + 1) * 128],
                                    lhsT=gb[:, 0, :], rhs=S[:, 0, :],
                                    start=(j == 0), stop=(j == nch_t - 1))
                        # drain completed bank (4 tiles, or end of half)
                        last_in_half = (tl == NA - 1) or (tl == NT - 1)
                        if q == 3 or last_in_half:
                            nb_t = gt - bank_gt0 + 1
                            w = nb_t * 128
                            stg = stp.tile([128, 512], BF16, name="stg")
                            nc.scalar.activation(
                                stg[:, :w], bank[:, :w], ACTF.Copy)
                            tl0 = bank_gt0 % NT
                            if tl0 < NA:
                                nc.sync.dma_start(
                                    partA_d[r, :, tl0 * 128:tl0 * 128 + w],
                                    stg[:, :w])
                            else:
                                off = (tl0 - NA) * 128
                                nc.sync.dma_start(
                                    partB_d[r, :, off:off + w], stg[:, :w])
                    # after the last A-half call: launch RS-A
                    if c0 + nch == chbase[cfg.gt_order[C * NA - 1]] + \
                            CH[cfg.gt_order[C * NA - 1]]:
                        if "rs" not in skip:
                            nc.gpsimd.collective_compute(
                                "ReduceScatter", OP.add,
                                replica_groups=groups,
                                ins=[partA_d[:]], outs=[rsA_d[:]])

                if "rs" not in skip:
                    nc.gpsimd.collective_compute(
                        "ReduceScatter", OP.add, replica_groups=groups,
                        ins=[partB_d[:]], outs=[rsB_d[:]])

                # ---- tail: self-add, dinv[dst] scale, BN stats ----------
                # per half: add self term (PE transpose of m block), then
                # fused (agg*dinv -> sum) and (agg*agg -> sum) DVE passes.
                for half, (t0, t1, w0) in enumerate(
                        ((0, NA, 0), (NA, NT, WA))):
                    rs_d = rsA_d if half == 0 else rsB_d
                    w = (t1 - t0) * 128
                    nc.sync.dma_start(rs_sb[:, w0:w0 + w], rs_d[:])
                    for tl in range(t0, t1):
                        ps2 = pmp.tile([128, 128], F32, name="pm")
                        nc.tensor.matmul(
                            ps2[:], lhsT=m_sb[:, tl, :],
                            rhs=Sself[:, tl, :], start=True, stop=True)
                        cl = slice(tl * 128, (tl + 1) * 128)
                        nc.vector.tensor_tensor(agg[:, cl], rs_sb[:, cl],
                                                ps2[:], OP.add)
                    hs = slice(w0, w0 + w)
                    nc.vector.tensor_tensor(
                        agg[:, hs], agg[:, hs], dinv_fm[:, hs], OP.mult)
                    nc.scalar.square(sq[:, :w], agg[:, hs])
                    nc.vector.tensor_reduce(
                        stat4[:, 2 * half:2 * half + 1], agg[:, hs],
                        axis=AX.X, op=OP.add)
                    nc.vector.tensor_reduce(
                        stat4[:, 2 * half + 1:2 * half + 2], sq[:, :w],
                        axis=AX.X, op=OP.add)

                nc.sync.dma_start(stat_in_d[:], stat4[:])
                if "ar" not in skip:
                    nc.gpsimd.collective_compute(
                        "AllGather", OP.bypass, replica_groups=groups,
                        ins=[stat_in_d[:]], outs=[stat_out_d[:]])
                nc.sync.dma_start(
                    statr[:], stat_out_d[:].rearrange("r p s -> p r s"))

                # stats2[s] = sum_r statr[r, s]
                nc.vector.tensor_reduce(
                    stats2[:], statr[:].rearrange("p r s -> p s r"),
                    axis=AX.X, op=OP.add)
                # mu = (S1A+S1B)/N; msq = (S2A+S2B)/N
                mu = prm[:, 0:1]
                msq = prm[:, 1:2]
                var = prm[:, 2:3]
                rsd = prm[:, 3:4]
                a_ = prm[:, 4:5]
                b_ = prm[:, 5:6]
                nc.vector.tensor_tensor(
                    prm[:, 0:2], stats2[:, 0:2], stats2[:, 2:4], OP.add)
                nc.vector.tensor_scalar(
                    out=prm[:, 0:2], in0=prm[:, 0:2], scalar1=inv_n,
                    scalar2=None, op0=OP.mult)
                nc.vector.tensor_tensor(var, mu, mu, OP.mult)
                nc.vector.tensor_tensor(var, msq, var, OP.subtract)
                nc.vector.tensor_scalar(
                    out=var, in0=var, scalar1=float(cfg.eps), scalar2=None,
                    op0=OP.add)
                nc.vector.reciprocal(rsd, var)
                nc.scalar.sqrt(rsd, rsd)
                nc.vector.tensor_tensor(a_, rsd, gammaT[:, l:l + 1], OP.mult)
                nc.vector.tensor_tensor(b_, mu, a_, OP.mult)
                nc.vector.tensor_tensor(b_, betaT[:, l:l + 1], b_, OP.subtract)

                # o = relu(a*agg + b)
                nc.scalar.activation(
                    o_fm[:], agg[:], ACTF.Relu, bias=b_, scale=a_)
                nc.gpsimd.dma_start(o_out_t[l], o_fm[:])

    nc.compile()
    return nc


# ----------------------------------------------------------------------------
# Entry point
# ----------------------------------------------------------------------------

_CACHE = {}


def kernel(x, edge_index, Ws, bs, gammas, betas):
    import numpy as _np
    from concourse.bass_utils import run_bass_kernel_spmd

    x = _np.asarray(x, dtype=_np.float32)
    edge_index = _np.asarray(edge_index)
    Ws = _np.asarray(Ws, dtype=_np.float32)
    gammas = _np.asarray(gammas, dtype=_np.float32)
    betas = _np.asarray(betas, dtype=_np.float32)

    cfg, per_core = preprocess(x, edge_index, Ws, gammas, betas, C=8)
    key = (cfg.NT, cfg.CHT, tuple(cfg.CH), tuple(cfg.gt_order))
    if key not in _CACHE:
        _CACHE.clear()
        _CACHE[key] = build_nc(cfg)
    nc = _CACHE[key]
    in_maps = [{k: _np.ascontiguousarray(v) for k, v in d.items()}
               for d in per_core]
    res = run_bass_kernel_spmd(nc, in_maps, core_ids=list(range(cfg.C)))
    core_outs = [res.results[c]["o_out"].reshape(cfg.L, 128, cfg.NTP)
                 for c in range(cfg.C)]
    return assemble_output(cfg, x, core_outs)
